# revision 1
# baseline (speedup 1.0000x reference)
"""CARAFE kernel for 8 TRN2 NeuronCores (Bass/Tile, SPMD).

Math (see reference):
  k0   = w_comp @ x + b_comp                 (64, 32, 32)      1x1 conv
  kc   = w_ker (*) k0 + b_ker                (102400, 32, 32)  3x3 conv, pad 1
  k    = softmax(kc.reshape(4, 25600, H, W), axis=1)
  ksum = k.sum(axis=1)                       (4, 32, 32)       == S/S (==1+eps)
  out  = (x[:, :, None] * ksum[:, None]).reshape(1, 256, 64, 64)

Sharding: tensor-parallel over the 102400 conv output channels, 12800 per
core. Each softmax group (25600 chans) spans cores (2s, 2s+1); group sums
are combined with a pairwise AllReduce. Core 2s+h computes the final
output for scale-group s, channel half h (128 of 256 x-channels).

Device layout choices:
  * The 3x3 conv is a matmul with contraction K = 64*9 (+1 bias row) = 577,
    M = 1024 pixels (PSUM partitions), N = 12800 channels (free dim).
    Channels on the free dim let ScalarE's Exp produce per-pixel partial
    softmax sums via accum_out for free.
  * No materialized im2col. The 9 conv taps are paired so each pair's two
    window offsets differ by a constant flat delta (+1 within an image row,
    +34 = one padded row). Three 128-partition copies of the padded
    compressed image serve as matmul lhsT directly via sliced window APs:
      T1 = [A; A<<1]  for tap pairs (0,1) (3,4) (6,7)
      T2 = [A; A<<34] for tap pair  (2,5)
      T3 = [A; ones]  for tap 8 + the bias row (K=65)
    The shifted upper halves are single contiguous SBUF->SBUF DMAs.
  * b_ker is folded into the matmul as the extra all-ones contraction row.
  * Conv compute in bf16: softmax sums are divided by themselves (ksum==1
    in exact arithmetic), so conv precision does not reach the output.
  * W is zero-padded to 640 contraction rows and blocked per (core, n-tile)
    on the host so each weight tile loads as one DMA of 128 partitions x 5KB
    contiguous (near-peak HBM bandwidth).
"""

import numpy as np

import concourse.bass as bass
import concourse.mybir as mybir
import concourse.tile as tile
from concourse import bacc
from concourse.bass_utils import run_bass_kernel_spmd

F32 = mybir.dt.float32
BF16 = mybir.dt.bfloat16
AF = mybir.ActivationFunctionType

# Problem constants
C, H, W = 256, 32, 32
CH = 64                   # compressed channels
NPIX = H * W              # 1024
OC_TOTAL = 102400
NCORES = 8
OC = OC_TOTAL // NCORES   # 12800 channels per core
KDIM = CH * 9             # 576
NK = 5                    # contraction k-tiles (4x128 + 65)
WROWS = NK * 128          # host-padded W rows (640)
NT = OC // 512            # 25 channel tiles of 512
MT = NPIX // 128          # 8 pixel tiles of 128
CHALF = C // 2            # 128 x-channels per core
PADW = W + 2              # 34

# tap pairing: k-tile kt holds taps (LOWTAP[kt], LOWTAP[kt]+delta) on
# partitions [0:64) and [64:128); T3 holds tap 8 + the bias ones row.
# tap t = (dh, dw) = (t // 3, t % 3), flat offset dh*34 + dw.
LOWTAP = [0, 3, 6, 2, 8]                  # kt -> low tap
TAPORDER = [0, 1, 3, 4, 6, 7, 2, 5, 8]    # W row grouping (matches pairs)


def build():
    nc = bacc.Bacc("TRN2", target_bir_lowering=False, debug=False,
                   num_devices=NCORES)

    xf = nc.dram_tensor("xf", [C, NPIX], BF16, kind="ExternalInput")
    xt = nc.dram_tensor("xt", [NPIX, CHALF], F32, kind="ExternalInput")
    wc = nc.dram_tensor("wc", [C, CH], BF16, kind="ExternalInput")
    bc = nc.dram_tensor("bc", [CH, 1], F32, kind="ExternalInput")
    wk = nc.dram_tensor("wk", [NT, 128, NK, 512], BF16, kind="ExternalInput")
    out = nc.dram_tensor("out", [NPIX, CHALF], F32, kind="ExternalOutput")
    sdbg = nc.dram_tensor("sdbg", [128, MT], F32, kind="ExternalOutput")

    with tile.TileContext(nc) as tc:
        with (
            tc.tile_pool(name="const", bufs=1) as const,
            tc.tile_pool(name="wpool", bufs=8) as wpool,
            tc.tile_pool(name="ppool", bufs=8, space="PSUM") as ppool,
            tc.tile_pool(name="epool", bufs=4) as epool,
            tc.tile_pool(name="dram", bufs=1, space="DRAM") as dram,
        ):
            def load_wt(n):
                # W is host-blocked per n-tile: 128 partitions x 5KB
                # contiguous, so one DMA runs at near-peak bandwidth
                wt = wpool.tile([128, NK, 512], BF16, tag="wt", name=f"wt_{n}")
                nc.sync.dma_start(wt[:], wk.ap()[n])
                return wt

            # ---- constants / staging (W n=0 hoisted ahead) ----
            wc_sb = const.tile([128, 2, CH], BF16)
            nc.sync.dma_start(wc_sb[:], wc.ap().rearrange("(k p) m -> p k m", p=128))
            bc_sb = const.tile([CH, 1], F32)
            nc.sync.dma_start(bc_sb[:], bc.ap())
            x_r = xf.ap().rearrange("(k p) n -> p k n", p=128)
            x_sb = const.tile([128, 2, NPIX], BF16)
            nc.sync.dma_start(x_sb[:, 0, :], x_r[:, 0, :])
            nc.sync.dma_start(x_sb[:, 1, :], x_r[:, 1, :])
            wts = {0: load_wt(0)}
            xt_sb = const.tile([128, MT, CHALF], F32)

            # padded-image composite tiles (halo zeros via memset; the upper
            # halves of T1/T2 are fully overwritten by the shift DMAs)
            T1 = const.tile([128, PADW, PADW], BF16)
            T2 = const.tile([128, PADW, PADW], BF16)
            T3 = const.tile([128, PADW, PADW], BF16)
            nc.vector.memset(T1[:], 0.0)
            nc.vector.memset(T3[0:64], 0.0)
            nc.vector.memset(T3[64:65], 1.0)
            nc.gpsimd.memset(T2[:], 0.0)

            # ---- compress conv: k0 = w_comp @ x + b_comp ----
            for nh in range(2):
                cps = ppool.tile([128, 512], F32, tag="ps", name=f"cps_{nh}")
                for kt in range(2):
                    nc.tensor.matmul(
                        cps[0:CH, :],
                        lhsT=wc_sb[:, kt, :],
                        rhs=x_sb[:, kt, nh * 512:(nh + 1) * 512],
                        start=(kt == 0), stop=(kt == 1),
                    )
                # evict (16 image rows per half) into T1's interior, + bias
                nc.scalar.activation(
                    T1[0:CH, 1 + nh * 16:1 + (nh + 1) * 16, 1:W + 1],
                    cps[0:CH, :].rearrange("p (a b) -> p a b", a=16),
                    AF.Identity, bias=bc_sb[:],
                )
            # replicate A into T2/T3 lower halves (partition-aligned fast DMAs)
            nc.sync.dma_start(T2[0:64, 1:H + 1, :], T1[0:64, 1:H + 1, :])
            nc.gpsimd.dma_start(T3[0:64, 1:H + 1, :], T1[0:64, 1:H + 1, :])

            # shifted upper halves: one contiguous SBUF->SBUF DMA each
            flat1 = T1[:].rearrange("p a b -> p (a b)")
            nc.sync.dma_start(flat1[64:128, 0:PADW * PADW - 1],
                              flat1[0:64, 1:PADW * PADW])
            flat2 = T2[:].rearrange("p a b -> p (a b)")
            nc.sync.dma_start(flat2[64:128, 0:PADW * PADW - PADW],
                              flat2[0:64, PADW:PADW * PADW])

            def lhsT_ap(kt, m, j):
                # one 32-pixel image row (single free dim) for col-tile j
                T = (T1, T1, T1, T2, T3)[kt]
                dh, dw = LOWTAP[kt] // 3, LOWTAP[kt] % 3
                kk = 128 if kt < 4 else 65
                r = dh + 4 * m + j
                return T[0:kk, r:r + 1, dw:dw + W]

            # ---- big conv + exp + per-pixel partial sums ----
            spart = const.tile([128, MT * NT], F32)     # (pix, m*NT+n)
            for n in range(NT):
                wt = wts.pop(n) if n in wts else load_wt(n)
                if n == 2:
                    # x^T load (only needed by the tail) off the startup path
                    nc.gpsimd.dma_start(
                        xt_sb[:], xt.ap().rearrange("(m p) c -> p m c", p=128))
                for mg in range(2):
                    pts = [
                        ppool.tile([128, 512], F32, tag="ps", name=f"ps_{n}_{mg}_{i}")
                        for i in range(4)
                    ]
                    for kt in range(NK):
                        kk = 128 if kt < 4 else 65
                        for mi in range(4):
                            m = mg * 4 + mi
                            for j in range(4):
                                nc.tensor.matmul(
                                    pts[mi][32 * j:32 * (j + 1), :],
                                    lhsT=lhsT_ap(kt, m, j),
                                    rhs=wt[0:kk, kt, :],
                                    start=(kt == 0), stop=(kt == NK - 1),
                                    tile_position=(0, 32 * j),
                                )
                    for mi in range(4):
                        m = mg * 4 + mi
                        et = epool.tile([128, 512], F32, tag="et")
                        nc.scalar.activation(
                            et[:], pts[mi][:], AF.Exp,
                            accum_out=spart[:, m * NT + n:m * NT + n + 1],
                        )

            # ---- per-core softmax sums -> pairwise AllReduce ----
            S = const.tile([128, MT], F32)
            for m in range(MT):
                nc.vector.tensor_reduce(
                    S[:, m:m + 1], spart[:, m * NT:(m + 1) * NT],
                    axis=mybir.AxisListType.X, op=mybir.AluOpType.add,
                )
            s_in = dram.tile([128, MT], F32)
            s_out = dram.tile([128, MT], F32)
            nc.sync.dma_start(s_in[:], S[:])
            nc.gpsimd.collective_compute(
                "AllReduce", mybir.AluOpType.add,
                replica_groups=[[0, 1], [2, 3], [4, 5], [6, 7]],
                ins=[s_in[:]], outs=[s_out[:]],
            )
            Sg = const.tile([128, MT], F32)
            nc.sync.dma_start(Sg[:], s_out[:])
            nc.sync.dma_start(sdbg.ap(), Sg[:])

            # ---- ksum = S/S ; out = x^T * ksum ----
            rec = const.tile([128, MT], F32)
            nc.vector.reciprocal(rec[:], Sg[:])
            ks = const.tile([128, MT], F32)
            nc.vector.tensor_mul(ks[:], Sg[:], rec[:])
            ot = const.tile([128, MT, CHALF], F32)
            for m in range(MT):
                nc.vector.tensor_scalar_mul(
                    ot[:, m, :], xt_sb[:, m, :], ks[:, m:m + 1],
                )
            nc.sync.dma_start(out.ap().rearrange("(m p) c -> p m c", p=128), ot[:])

    nc.compile()
    return nc


_NC = None


def _get_nc():
    global _NC
    if _NC is None:
        _NC = build()
    return _NC


def prep_inputs(x, w_comp, b_comp, w_ker, b_ker):
    import ml_dtypes
    x = np.asarray(x, dtype=np.float32)
    w_comp = np.asarray(w_comp, dtype=np.float32)
    b_comp = np.asarray(b_comp, dtype=np.float32)
    w_ker = np.asarray(w_ker, dtype=np.float32)
    b_ker = np.asarray(b_ker, dtype=np.float32)
    xf = np.ascontiguousarray(x.reshape(C, NPIX)).astype(ml_dtypes.bfloat16)
    xt_full = np.ascontiguousarray(x.reshape(C, NPIX).astype(np.float32).T)
    wcT = np.ascontiguousarray(w_comp.reshape(CH, C).T).astype(ml_dtypes.bfloat16)
    bcr = np.ascontiguousarray(b_comp.reshape(CH, 1), dtype=np.float32)
    wt = np.zeros((WROWS, OC_TOTAL), dtype=ml_dtypes.bfloat16)
    w9 = w_ker.reshape(OC_TOTAL, CH, 9)[:, :, TAPORDER]     # (O, 64, 9 slots)
    wt[:KDIM] = w9.transpose(2, 1, 0).reshape(KDIM, OC_TOTAL)
    wt[KDIM] = b_ker                                        # row 576 = bias
    # per-core, per-n-tile contiguous blocks: (NT, 128, NK, 512)
    wtb = wt.reshape(NK, 128, NCORES, NT, 512).transpose(2, 3, 1, 0, 4)
    in_maps = []
    for core in range(NCORES):
        h = core % 2
        in_maps.append({
            "xf": xf,
            "xt": np.ascontiguousarray(xt_full[:, h * CHALF:(h + 1) * CHALF]),
            "wc": wcT,
            "bc": bcr,
            "wk": np.ascontiguousarray(wtb[core]),
        })
    return in_maps


def assemble(results, x):
    full = np.empty((C, 2 * H, 2 * W), dtype=np.float32)
    for core in range(NCORES):
        s, h = core // 2, core % 2
        blk = results[core]["out"]                            # (1024, 128)
        full[h * CHALF:(h + 1) * CHALF, s * 16:(s + 1) * 16, :] = (
            blk.T.reshape(CHALF, 16, 64)
        )
    return full.reshape(1, C, 2 * H, 2 * W)


def run(in_maps, trace=False, **kw):
    nc = _get_nc()
    return run_bass_kernel_spmd(nc, in_maps, list(range(NCORES)), trace=trace, **kw)


def kernel(x, w_comp, b_comp, w_ker, b_ker):
    in_maps = prep_inputs(x, w_comp, b_comp, w_ker, b_ker)
    res = run(in_maps)
    return assemble(res.results, x)



# revision 6
# speedup vs baseline: 1.7238x; 1.7238x over previous
"""CARAFE kernel for 8 TRN2 NeuronCores (Bass/Tile, SPMD).

Math (see reference):
  k0   = w_comp @ x + b_comp                 (64, 32, 32)      1x1 conv
  kc   = w_ker (*) k0 + b_ker                (102400, 32, 32)  3x3 conv, pad 1
  k    = softmax(kc.reshape(4, 25600, H, W), axis=1)
  ksum = k.sum(axis=1)                       (4, 32, 32)       == S/S (==1+eps)
  out  = (x[:, :, None] * ksum[:, None]).reshape(1, 256, 64, 64)

Sharding: core c = (g, h) with g = c//2 (softmax/scale group), h = c%2
(pixel half: image rows 16h..16h+16).  Each core computes its group's
FULL 25600 conv channels for its OWN 512 pixels, so the softmax group
sum S is core-local -- no collective at all (the baseline's pairwise
AllReduce cost ~29us of pure tail latency).

Device layout choices:
  * The 3x3 conv is evaluated in fp8e4 with DoubleRow perf mode: each
    matmul contracts 256 im2col rows (2 fp8 weights/cell), streaming
    N=512 channels.  Contraction split: ktile A = taps 0-3, ktile B =
    taps 4-7 (DoubleRow, 256 rows each), tap 8 as a 64-row fp8 tail.
  * Weights are scaled x16 on the host before fp8 quantization (their
    0.05 sigma sits in e4m3's subnormal range); the Exp eviction's
    activation scale of 1/16 undoes it exactly.  Softmax sums are
    divided by themselves (ksum == 1 in exact arithmetic), so conv
    precision does not reach the output.
  * im2col without materialization: the compressed image (18 rows incl
    halo) lives in a 23-row x 32-col zero-framed fp8 strip; each tap is
    a flat-shifted SBUF->SBUF copy of it (shift (dh-1)*32 + (dw-1)).
    Row-crossing leaks of the dw!=1 shifts are zeroed with tiny column
    memsets, so the conv is exact.  A 128-pixel matmul window (4 image
    rows) is then a single contiguous 128B slice -- flat 3D DoubleRow
    lhsT APs [128, 2, 128].
  * The stationary operand is the image window, shared by consecutive
    matmuls (channel-tile pairs), keeping LDWEIGHTS off the critical
    path; psum tiles span 2 banks ([128, 1024]) holding a channel-tile
    pair, halving ScalarE eviction instruction count.
  * tap-8 tails for a channel-tile pair are packed as two concurrent
    row-tiled K=64 matmuls (array rows 0-63 / 64-127), costing ~one
    matmul slot instead of two.  b_ker is all zeros for this problem's
    setup_inputs; a separate K=65 variant (ones row + b_ker contraction
    row) is built lazily if any nonzero b_ker is ever passed.
  * Exp eviction on ScalarE produces the per-pixel partial softmax sums
    via accum_out for free; a final DVE reduce + S*(1/S) + x multiply
    produces the output block.
"""

import numpy as np

import concourse.bass as bass
import concourse.mybir as mybir
import concourse.tile as tile
from concourse import bacc
from concourse.bass_utils import run_bass_kernel_spmd

F32 = mybir.dt.float32
BF16 = mybir.dt.bfloat16
FP8 = mybir.dt.float8e4
AF = mybir.ActivationFunctionType
DR = mybir.MatmulPerfMode.DoubleRow

# Problem constants
C, H, W = 256, 32, 32
CH = 64                    # compressed channels
SC = 2                     # upsample scale
OC_TOTAL = 102400
NCORES = 8
GCH = OC_TOTAL // 4        # 25600 channels per softmax group (= per core)
NT = GCH // 512            # 50 channel tiles of 512
NPAIR = NT // 2            # 25 channel-tile pairs
PIX = 512                  # pixels per core (16 image rows)
MT = PIX // 128            # 4 pixel tiles of 128 (4 image rows each)
HLOC = 18                  # local k0 rows incl 1-row halo each side
NLOC = HLOC * W            # 576 compress-conv pixels
FROWS = 23                 # zero-framed strip rows
FRAME = FROWS * W          # 736 bytes per image copy (div by 16)
WSCALE = 16.0              # host weight scale, undone by Exp's 1/16

# frame row f holds k0 local row f-3 (local rows -1..16 at f=2..19)
EV0 = 2 * W                # eviction start: flat offset of frame row 2
# matmul window for m-tile mt: local rows 4mt..4mt+3 -> frame rows
# 4mt+3..4mt+6 -> flat [32*(4mt+3), +128)
def WOFF(mt):
    return W * (4 * mt + 3)

# tap t = (dh, dw) = (t//3, t%3); copy shift = (dh-1)*32 + (dw-1)
# DoubleRow ktile A: (i, phalf) -> tap [[0, 1], [2, 3]]; B: [[4, 5], [6, 7]]
A_TAPS = [[0, 1], [2, 3]]
B_TAPS = [[4, 5], [6, 7]]


def build(with_bias=False):
    nc = bacc.Bacc("TRN2", target_bir_lowering=False, debug=False,
                   num_devices=NCORES)

    xf = nc.dram_tensor("xf", [C, NLOC], BF16, kind="ExternalInput")
    xt = nc.dram_tensor("xt", [PIX, C], F32, kind="ExternalInput")
    wc = nc.dram_tensor("wc", [C, CH], BF16, kind="ExternalInput")
    bc = nc.dram_tensor("bc", [CH, 1], F32, kind="ExternalInput")
    wk = nc.dram_tensor("wk", [NT, 128, 2, 2, 512], FP8, kind="ExternalInput")
    if with_bias:
        wkt = nc.dram_tensor("wkt", [NT, 65, 512], FP8, kind="ExternalInput")
    else:
        wkt = nc.dram_tensor("wkt", [NPAIR, 128, 512], FP8, kind="ExternalInput")
    out = nc.dram_tensor("out", [PIX, C], F32, kind="ExternalOutput")
    sdbg = nc.dram_tensor("sdbg", [128, MT], F32, kind="ExternalOutput")

    with tile.TileContext(nc) as tc:
        with (
            tc.tile_pool(name="const", bufs=1) as const,
            tc.tile_pool(name="wpool", bufs=6) as wpool,
            tc.tile_pool(name="tpool", bufs=3) as tpool,
            tc.tile_pool(name="ppool", bufs=4, space="PSUM") as ppool,
            tc.tile_pool(name="epool", bufs=2) as epool,
        ):
            def load_wt(n):
                wt = wpool.tile([128, 2, 2, 512], FP8, tag="wt", name=f"wt_{n}")
                nc.sync.dma_start(wt[:], wk.ap()[n])
                return wt

            def load_tail(pair):
                shape = [65 if with_bias else 128, 512]
                tt = tpool.tile(shape, FP8, tag="tt", name=f"tt_{pair}")
                nc.sync.dma_start(tt[:], wkt.ap()[pair])
                return tt

            # ---- input staging; weight prefetch ahead of everything ----
            wts = {0: load_wt(0), 1: load_wt(1)}
            tts = {0: load_tail(0)} if not with_bias else {0: load_tail(0),
                                                           1: load_tail(1)}
            wc_sb = const.tile([128, 2, CH], BF16)
            nc.sync.dma_start(wc_sb[:], wc.ap().rearrange("(k p) m -> p k m", p=128))
            bc_sb = const.tile([CH, 1], F32)
            nc.sync.dma_start(bc_sb[:], bc.ap())
            x_sb = const.tile([128, 2, NLOC], BF16)
            nc.gpsimd.dma_start(x_sb[:], xf.ap().rearrange("(k p) n -> p k n", p=128))
            xt_sb = const.tile([128, MT, C], F32)
            nc.gpsimd.dma_start(xt_sb[:], xt.ap().rearrange("(m p) c -> p m c", p=128))

            # image strips: U1 = ktile A (taps 0-3), U2 = ktile B (taps 4-7),
            # V8 = tap 8 duplicated on both partition halves.
            U1 = const.tile([128, 2, FRAME], FP8)
            U2 = const.tile([128, 2, FRAME], FP8)
            V8 = const.tile([128, FRAME], FP8)
            # base strip = U2[0:64, 0] (tap 4, shift 0): zero its frame edges
            nc.vector.memset(U2[0:64, 0, 0:EV0], 0.0)
            nc.vector.memset(U2[0:64, 0, EV0 + NLOC:FRAME], 0.0)

            # ---- compress conv: k0 = w_comp @ x + b_comp, evict as fp8 ----
            for nh in range(2):
                cps = ppool.tile([128, 1024], F32, tag="ps", name=f"cps_{nh}")
                for kt in range(2):
                    nc.tensor.matmul(
                        cps[0:CH, 0:NLOC // 2],
                        lhsT=wc_sb[:, kt, :],
                        rhs=x_sb[:, kt, nh * (NLOC // 2):(nh + 1) * (NLOC // 2)],
                        start=(kt == 0), stop=(kt == 1),
                    )
                nc.scalar.activation(
                    U2[0:CH, 0, EV0 + nh * (NLOC // 2):EV0 + (nh + 1) * (NLOC // 2)],
                    cps[0:CH, 0:NLOC // 2],
                    AF.Identity, bias=bc_sb[:],
                )

            # ---- 9 flat shifted copies of the base strip ----
            base = U2[0:64, 0, :]
            CPY0, CPY1 = 2 * W, 21 * W          # dst copy extent [64, 672)
            def tapcopy(dst, t, q):
                s = (t // 3 - 1) * W + (t % 3 - 1)
                q.dma_start(dst[:, CPY0:CPY1], base[:, CPY0 + s:CPY1 + s])
            tapcopy(U1[0:64, 0, :], 0, nc.sync)
            tapcopy(U1[64:128, 0, :], 1, nc.gpsimd)
            tapcopy(U1[0:64, 1, :], 2, nc.scalar)
            tapcopy(U1[64:128, 1, :], 3, nc.sync)
            tapcopy(U2[64:128, 0, :], 5, nc.gpsimd)
            tapcopy(U2[0:64, 1, :], 6, nc.scalar)
            tapcopy(U2[64:128, 1, :], 7, nc.sync)
            tapcopy(V8[0:64, :], 8, nc.gpsimd)
            if with_bias:
                nc.vector.memset(V8[64:65, :], 1.0)    # bias ones row
            else:
                tapcopy(V8[64:128, :], 8, nc.scalar)

            # zero the row-crossing leak columns (dw=0 -> col 0, dw=2 -> col 31)
            # over the window rows 3..18
            def colfix(strip, col):
                ap = strip.rearrange("p (r c) -> p r c", c=W)
                nc.vector.memset(ap[:, 3:19, col:col + 1], 0.0)
            colfix(U1[0:64, 0, :], 0)        # tap 0
            colfix(U1[0:64, 1, :], 31)       # tap 2
            colfix(U1[64:128, 1, :], 0)      # tap 3
            colfix(U2[64:128, 0, :], 31)     # tap 5
            colfix(U2[0:64, 1, :], 0)        # tap 6
            colfix(V8[0:64, :], 31)          # tap 8
            if not with_bias:
                colfix(V8[64:128, :], 31)    # tap 8 dup

            # ---- big conv + exp + per-pixel partial sums ----
            spart = const.tile([128, NPAIR * MT], F32)
            for pair in range(NPAIR):
                n0, n1 = 2 * pair, 2 * pair + 1
                wt0 = wts.pop(n0) if n0 in wts else load_wt(n0)
                wt1 = wts.pop(n1) if n1 in wts else load_wt(n1)
                if with_bias:
                    tt0 = tts.pop(n0) if n0 in tts else load_tail(n0)
                    tt1 = tts.pop(n1) if n1 in tts else load_tail(n1)
                else:
                    tt = tts.pop(pair) if pair in tts else load_tail(pair)
                for mt in range(MT):
                    w0, w1 = WOFF(mt), WOFF(mt) + 128
                    lhsA = U1[:, :, w0:w1]
                    lhsB = U2[:, :, w0:w1]
                    pt = ppool.tile([128, 1024], F32, tag="ps",
                                    name=f"pt_{pair}_{mt}")
                    nc.tensor.matmul(pt[:, 0:512], lhsT=lhsA, rhs=wt0[:, 0],
                                     start=True, stop=False, perf_mode=DR)
                    nc.tensor.matmul(pt[:, 512:1024], lhsT=lhsA, rhs=wt1[:, 0],
                                     start=True, stop=False, perf_mode=DR)
                    nc.tensor.matmul(pt[:, 0:512], lhsT=lhsB, rhs=wt0[:, 1],
                                     start=False, stop=False, perf_mode=DR)
                    nc.tensor.matmul(pt[:, 512:1024], lhsT=lhsB, rhs=wt1[:, 1],
                                     start=False, stop=False, perf_mode=DR)
                    if with_bias:
                        # ones row lives at V8[64] / tt[64]; K=65, serial
                        nc.tensor.matmul(pt[:, 0:512],
                                         lhsT=V8[0:65, w0:w1], rhs=tt0[:],
                                         start=False, stop=True)
                        nc.tensor.matmul(pt[:, 512:1024],
                                         lhsT=V8[0:65, w0:w1], rhs=tt1[:],
                                         start=False, stop=True)
                    else:
                        # packed K=64 tails: concurrent row-tiled matmuls
                        nc.tensor.matmul(pt[:, 0:512],
                                         lhsT=V8[0:64, w0:w1], rhs=tt[0:64, :],
                                         start=False, stop=True)
                        nc.tensor.matmul(pt[:, 512:1024],
                                         lhsT=V8[64:128, w0:w1], rhs=tt[64:128, :],
                                         start=False, stop=True)
                    et = epool.tile([128, 1024], BF16, tag="et")
                    idx = pair * MT + mt
                    nc.scalar.activation(
                        et[:], pt[:], AF.Exp, scale=1.0 / WSCALE,
                        accum_out=spart[:, idx:idx + 1],
                    )

            # ---- softmax group sums -> ksum = S/S -> out = x^T * ksum ----
            S = const.tile([128, MT], F32)
            for mt in range(MT):
                nc.vector.tensor_reduce(
                    S[:, mt:mt + 1], spart[:, mt::MT],
                    axis=mybir.AxisListType.X, op=mybir.AluOpType.add,
                )
            nc.sync.dma_start(sdbg.ap(), S[:])
            rec = const.tile([128, MT], F32)
            nc.vector.reciprocal(rec[:], S[:])
            ks = const.tile([128, MT], F32)
            nc.vector.tensor_mul(ks[:], S[:], rec[:])
            ot = const.tile([128, MT, C], F32)
            for mt in range(MT):
                nc.vector.tensor_scalar_mul(
                    ot[:, mt, :], xt_sb[:, mt, :], ks[:, mt:mt + 1],
                )
            nc.sync.dma_start(out.ap().rearrange("(m p) c -> p m c", p=128), ot[:])

    nc.compile()
    return nc


_NC = {}


def _get_nc(with_bias=False):
    if with_bias not in _NC:
        _NC[with_bias] = build(with_bias)
    return _NC[with_bias]


def prep_inputs(x, w_comp, b_comp, w_ker, b_ker):
    import ml_dtypes
    E4 = ml_dtypes.float8_e4m3
    x = np.asarray(x, dtype=np.float32).reshape(C, H, W)
    w_comp = np.asarray(w_comp, dtype=np.float32)
    b_comp = np.asarray(b_comp, dtype=np.float32)
    w_ker = np.asarray(w_ker, dtype=np.float32)
    b_ker = np.asarray(b_ker, dtype=np.float32)
    with_bias = bool(np.any(b_ker))

    xp = np.zeros((C, H + 2, W), np.float32)
    xp[:, 1:H + 1] = x
    wcT = np.ascontiguousarray(w_comp.reshape(CH, C).T).astype(ml_dtypes.bfloat16)
    bcr = np.ascontiguousarray(b_comp.reshape(CH, 1), dtype=np.float32)

    # weights: x16 scale, fp8e4, grouped [nt, p=hi*64+ci, kt, i, n]
    w9 = (w_ker.reshape(OC_TOTAL, CH, 9) * WSCALE).astype(E4)
    bk16 = (b_ker * WSCALE).astype(E4)

    in_maps = []
    for core in range(NCORES):
        g, h = core // 2, core % 2
        xf = np.ascontiguousarray(
            xp[:, 16 * h:16 * h + HLOC].reshape(C, NLOC)).astype(ml_dtypes.bfloat16)
        xtc = np.ascontiguousarray(
            x.reshape(C, H * W)[:, PIX * h:PIX * (h + 1)].T)
        a = w9[GCH * g:GCH * (g + 1)].reshape(NT, 512, CH, 9)
        wkc = np.empty((NT, 128, 2, 2, 512), E4)
        for kt, taps in enumerate((A_TAPS, B_TAPS)):
            for i in range(2):
                for hi in range(2):
                    wkc[:, 64 * hi:64 * (hi + 1), kt, i, :] = (
                        a[:, :, :, taps[i][hi]].transpose(0, 2, 1))
        t8 = a[:, :, :, 8].transpose(0, 2, 1)          # (NT, 64, 512)
        if with_bias:
            wktc = np.empty((NT, 65, 512), E4)
            wktc[:, 0:64] = t8
            wktc[:, 64] = bk16[GCH * g:GCH * (g + 1)].reshape(NT, 512)
        else:
            wktc = np.ascontiguousarray(t8.reshape(NPAIR, 128, 512))
        in_maps.append({
            "xf": xf,
            "xt": xtc,
            "wc": wcT,
            "bc": bcr,
            "wk": np.ascontiguousarray(wkc),
            "wkt": wktc,
        })
    return in_maps, with_bias


def assemble(results):
    full = np.empty((C, 4, H, W), dtype=np.float32)
    for core in range(NCORES):
        g, h = core // 2, core % 2
        blk = results[core]["out"]                     # (512, 256)
        full[:, g, 16 * h:16 * (h + 1), :] = blk.T.reshape(C, 16, W)
    return full.reshape(1, C, SC * H, SC * W)


def run(in_maps, with_bias=False, trace=False, **kw):
    nc = _get_nc(with_bias)
    return run_bass_kernel_spmd(nc, in_maps, list(range(NCORES)), trace=trace, **kw)


def kernel(x, w_comp, b_comp, w_ker, b_ker):
    in_maps, with_bias = prep_inputs(x, w_comp, b_comp, w_ker, b_ker)
    res = run(in_maps, with_bias)
    return assemble(res.results)
